# revision 21
# baseline (speedup 1.0000x reference)
"""Multi-head attention (B=4, S=2048, D=1024, H=16, causal) on 8 TRN2 cores.

Sharding: core c -> (batch b = c//2, head-group g = c%2 of 8 heads).
Each core: projections for its 8 heads (column-split Wq/Wk/Wv), causal
attention, partial output projection (row-split Wo), output transposed
[D, S]. Host sums the two partials per batch, transposes, adds bo.

v2: bf16 datapath (PSUM accum stays f32), qc-outer attention loop with
the per-qc tail (recip, normalize, output projection, out-DMA)
software-pipelined into the next qc's attention, merged A|B exp
activations, chunked input DMA ordered by first use. Goal: keep the PE
warm (HAM) and the Activation engine (exp) ~100% busy.
"""

from collections import deque
from contextlib import ExitStack

import numpy as np
import ml_dtypes

import concourse.bass as bass
import concourse.tile as tile
from concourse import bacc, mybir
from concourse.bass_utils import run_bass_kernel_spmd

F32 = mybir.dt.float32
F32R = mybir.dt.float32r
BF16 = mybir.dt.bfloat16
EXP = mybir.ActivationFunctionType.Exp
COPY = mybir.ActivationFunctionType.Copy

B, S, D, H = 4, 2048, 1024, 16
HD = D // H          # 64
DL = D // 2          # 512 local douts per core
NT = DL // 128       # 4 dout tiles / head pairs
NR = S // 128        # 16 row tiles
NQ = S // 512        # 4 query chunks
NDIN = D // 128      # 8 din tiles


def build_nc():
    nc = bacc.Bacc("TRN2", target_bir_lowering=False, debug=False, num_devices=8)

    qT = nc.dram_tensor("qT", [D, S], BF16, kind="ExternalInput").ap()
    kT = nc.dram_tensor("kT", [D, S], BF16, kind="ExternalInput").ap()
    vT = nc.dram_tensor("vT", [D, S], BF16, kind="ExternalInput").ap()
    Wq_s = nc.dram_tensor("Wq_s", [D, DL], BF16, kind="ExternalInput").ap()
    Wk_s = nc.dram_tensor("Wk_s", [D, DL], BF16, kind="ExternalInput").ap()
    Wv_s = nc.dram_tensor("Wv_s", [D, DL], BF16, kind="ExternalInput").ap()
    Wo_s = nc.dram_tensor("Wo_s", [DL, D], BF16, kind="ExternalInput").ap()
    bq_s = nc.dram_tensor("bq_s", [DL, 1], F32, kind="ExternalInput").ap()
    bk_s = nc.dram_tensor("bk_s", [DL, 1], F32, kind="ExternalInput").ap()
    bv_bc = nc.dram_tensor("bv_bc", [128, DL], BF16, kind="ExternalInput").ap()
    E_in = nc.dram_tensor("E_in", [8, DL], BF16, kind="ExternalInput").ap()
    # output transposed: [D, S]
    out_p = nc.dram_tensor("out_partial", [D, S], BF16, kind="ExternalOutput").ap()

    with tile.TileContext(nc) as tc, ExitStack() as ctx:
        # ---------------- persistent SBUF ----------------
        keep = ctx.enter_context(tc.tile_pool(name="keep", bufs=1))
        qwT = [keep.tile([128, S], BF16, tag=f"qwT{t}", name=f"qwT{t}") for t in range(NT)]
        kwT = [keep.tile([128, S], BF16, tag=f"kwT{t}", name=f"kwT{t}") for t in range(NT)]
        vw = [keep.tile([128, 8 * 65], BF16, tag=f"vw{r}", name=f"vw{r}") for r in range(NR)]
        atn = [keep.tile([128, S], BF16, tag=f"atn{t}", name=f"atn{t}") for t in range(NT)]
        bias_q = keep.tile([128, NT], F32, tag="bias_q")  # col t = bq tile t
        bias_k = keep.tile([128, NT], F32, tag="bias_k")
        bv_sb = keep.tile([128, DL], BF16, tag="bv_sb")
        E_sb = keep.tile([8, DL], BF16, tag="E_sb", name="E_sb")
        wo_sb = [keep.tile([128, D], BF16, tag=f"wo{t}", name=f"wo{t}") for t in range(NT)]

        def load_small():
            # one DMA per bias: [DL,1] dram -> [128, NT] sbuf (col t = tile t)
            nc.gpsimd.dma_start(bias_q[:, 0:NT],
                                bq_s[:].rearrange("(t p) one -> p (t one)", p=128))
            nc.gpsimd.dma_start(bias_k[:, 0:NT],
                                bk_s[:].rearrange("(t p) one -> p (t one)", p=128))
            nc.gpsimd.dma_start(bv_sb[:], bv_bc)
            nc.gpsimd.dma_start(E_sb[:], E_in)

        warm = keep.tile([1, NT], F32, tag="warm", name="warm")
        onesT = keep.tile([1, 128], BF16, tag="onesT", name="onesT")
        nc.vector.memset(onesT[:], 1.0)

        # ones columns in vw (col 64 of each head's 65-wide slice)
        for r in range(NR):
            ones_ap = vw[r][:].rearrange("p (h e) -> p h e", e=65)[:, :, 64:65]
            nc.vector.memset(ones_ap, 1.0)

        # ---------------- projections ----------------
        wtp = ctx.enter_context(tc.tile_pool(name="wt", bufs=2))
        slp = ctx.enter_context(tc.tile_pool(name="slab", bufs=2))
        with tc.tile_pool(name="pps", bufs=6, space="PSUM") as pps:

            def load_w(W):
                w_sb = []
                for dn in range(NDIN):
                    w = wtp.tile([128, DL], BF16, tag=f"w{dn}", name=f"w{dn}")
                    nc.gpsimd.dma_start(w[:], W[128 * dn:128 * (dn + 1), :])
                    w_sb.append(w)
                return w_sb

            def load_slab(xT, nchunk=2):
                # first-use-ordered chunks: first chunk small so the first
                # matmul starts early, remainder in one DMA per din tile
                sl = []
                for dn in range(NDIN):
                    s_ = slp.tile([128, S], BF16, tag=f"sl{dn}", name=f"sl{dn}")
                    sl.append(s_)
                for rc in range(NQ):
                    for dn in range(NDIN):
                        nc.gpsimd.dma_start(
                            sl[dn][:, 512 * rc:512 * (rc + 1)],
                            xT[128 * dn:128 * (dn + 1), 512 * rc:512 * (rc + 1)])
                return sl

            def proj_T(w_sb, sl, bias_t, dst):
                # dst[t][:, rc] = tile t of (x @ W).T + bias (douts on partitions)
                for rc in range(NQ):
                    for t in range(NT):
                        ps = pps.tile([128, 512], F32, tag="pp", name="pp")
                        for dn in range(NDIN):
                            nc.tensor.matmul(
                                ps[:],
                                w_sb[dn][:, 128 * t:128 * (t + 1)],
                                sl[dn][:, 512 * rc:512 * (rc + 1)],
                                start=(dn == 0), stop=(dn == NDIN - 1))
                        nc.vector.tensor_scalar_add(
                            dst[t][:, 512 * rc:512 * (rc + 1)],
                            ps[:], bias_t[:, t:t + 1])

            wq_sb = load_w(Wq_s)
            slq = load_slab(qT)
            load_small()
            # preload the exp table set so attention's first ACTIVATE
            # doesn't eat the ~2.7us ACT_TABLE_LOAD
            nc.scalar.activation(warm[:], bias_q[0:1, 0:NT], EXP)
            proj_T(wq_sb, slq, bias_q, qwT)

            wk_sb = load_w(Wk_s)
            slk = load_slab(kT)
            proj_T(wk_sb, slk, bias_k, kwT)

            # vw natural: [row, dout] with per-head ones column.
            # Only r0..7 here; r8..15 run interleaved into attention qc0/qc1
            # (they aren't needed before qc2) to smooth the proj->attention
            # transition and keep the PE HAM-warm.
            wv_sb = load_w(Wv_s)
            slv = load_slab(vT)
            bv3 = bv_sb[:].rearrange("p (h e) -> p h e", e=64)

            def v_group(r, pool, tag):
                ps = pool.tile([128, 512], F32, tag=tag, name="pp")
                for dn in range(NDIN):
                    nc.tensor.matmul(
                        ps[:],
                        slv[dn][:, 128 * r:128 * (r + 1)],
                        wv_sb[dn][:],
                        start=(dn == 0), stop=(dn == NDIN - 1))
                dst3 = vw[r][:].rearrange("p (h e) -> p h e", e=65)[:, :, 0:64]
                nc.vector.tensor_add(
                    dst3, ps[:].rearrange("p (h e) -> p h e", e=64), bv3)

            for r in range(NR // 2):
                v_group(r, pps, "pp")

            # Wo tiles: issue after v-slabs; arrive during attention(0)
            for t in range(NT):
                nc.gpsimd.dma_start(wo_sb[t][:], Wo_s[128 * t:128 * (t + 1), :])

        # ------------- attention + pipelined per-qc tail -------------
        with tc.tile_pool(name="scp", bufs=2, space="PSUM") as scp, \
             tc.tile_pool(name="atp", bufs=3, space="PSUM") as atp, \
             tc.tile_pool(name="pob", bufs=1, space="PSUM") as pob, \
             tc.tile_pool(name="prp", bufs=5) as prp, \
             tc.tile_pool(name="smp", bufs=2) as smp, \
             tc.tile_pool(name="osb", bufs=2) as osp:

            tail_q = deque()
            for r in range(NR // 2, NR):
                tail_q.append(
                    (lambda r=r: v_group(r, pob, "pobc")))

            def make_tail(qc, sums_qc):
                qf = slice(512 * qc, 512 * (qc + 1))
                if qc < NQ - 1:
                    recip_qc = smp.tile([8, 512], F32, tag="recip", name="recip")
                    recip_bf = smp.tile([8, 512], BF16, tag="recipb", name="recipb")

                def op_recip():
                    nc.vector.reciprocal_approx_fast(recip_qc[:], sums_qc[0:8, :])
                    nc.vector.tensor_copy(recip_bf[:], recip_qc[:])
                ops = [] if qc == NQ - 1 else [op_recip]

                def mk_norm(t):
                    def op():
                        bc = pob.tile([128, 512], F32, tag="pobc", name="bc")
                        nc.tensor.matmul(
                            bc[:], E_sb[:, 128 * t:128 * (t + 1)],
                            recip_bf[:],
                            start=True, stop=True)
                        nc.vector.tensor_mul(
                            atn[t][:, qf], atn[t][:, qf], bc[:])
                    return op
                if qc != NQ - 1:
                    ops += [mk_norm(t) for t in range(NT)]

                def mk_oproj(dc, pool, tag):
                    def op():
                        po = pool.tile([128, 512], F32, tag=tag, name="po")
                        for t in range(NT):
                            nc.tensor.matmul(
                                po[:],
                                wo_sb[t][:, 128 * dc:128 * (dc + 1)],
                                atn[t][:, qf],
                                start=(t == 0), stop=(t == NT - 1))
                        ob = osp.tile([128, 512], BF16, tag="ob", name="ob")
                        nc.vector.tensor_copy(ob[:], po[:])
                        if qc == NQ - 1:
                            # final tail: split across two queues to halve
                            # the exposed out-DMA latency
                            h = 512 * qc + 256
                            nc.sync.dma_start(
                                out_p[128 * dc:128 * (dc + 1), 512 * qc:h],
                                ob[:, 0:256])
                            nc.sync.dma_start(
                                out_p[128 * dc:128 * (dc + 1), h:512 * (qc + 1)],
                                ob[:, 256:512])
                        else:
                            nc.sync.dma_start(
                                out_p[128 * dc:128 * (dc + 1), qf], ob[:])
                    return op
                # last qc: scores pool is idle afterwards, use its slots to
                # double-buffer the output projection
                if qc == NQ - 1:
                    ops += [mk_oproj(dc, scp, "sc") for dc in range(NDIN)]
                else:
                    ops += [mk_oproj(dc, pob, "pobc") for dc in range(NDIN)]
                return ops

            # three-j software pipeline: PV of j trails the scores of j+3 so
            # the PE never serializes on exp at p/qc boundaries
            from collections import deque as _dq
            pv_lag = _dq()

            def mk_pv(qc, p, j, jmax, off, ATA, ATB, PR, sums_qc, qf):
                def op():
                    nc.tensor.matmul(
                        ATA[0:65, off:512],
                        vw[j][:, 65 * 2 * p:65 * 2 * p + 65],
                        PR[:, off:512],
                        start=(j == 0), stop=(j == jmax))
                    nc.tensor.matmul(
                        ATB[0:65, off:512],
                        vw[j][:, 65 * (2 * p + 1):65 * (2 * p + 1) + 65],
                        PR[:, 512 + off:1024],
                        start=(j == 0), stop=(j == jmax))
                    if j == jmax:
                        # drain head pair p (A-side first so its AT slot
                        # frees earliest; the next pair's PV waits on these
                        # through the pool rotation)
                        stgA = smp.tile([1, 512], F32, tag="stgA", name="stgA")
                        stgB = smp.tile([1, 512], F32, tag="stgB", name="stgB")
                        nc.vector.tensor_copy(atn[p][0:64, qf], ATA[0:64, :])
                        nc.vector.tensor_copy(stgA[:], ATA[64:65, :])
                        nc.vector.tensor_copy(atn[p][64:128, qf], ATB[0:64, :])
                        nc.vector.tensor_copy(stgB[:], ATB[64:65, :])
                        if qc == NQ - 1:
                            # last qc: normalize per head pair so only one
                            # pair's chain remains after the final PV
                            rA = smp.tile([1, 512], F32, tag="rA", name="rA")
                            rB = smp.tile([1, 512], F32, tag="rB", name="rB")
                            rAb = smp.tile([1, 512], BF16, tag="rAb", name="rAb")
                            rBb = smp.tile([1, 512], BF16, tag="rBb", name="rBb")
                            nc.vector.reciprocal_approx_fast(rA[:], stgA[:])
                            nc.vector.reciprocal_approx_fast(rB[:], stgB[:])
                            nc.vector.tensor_copy(rAb[:], rA[:])
                            nc.vector.tensor_copy(rBb[:], rB[:])

                            def norm_p(p=p):
                                bc = pob.tile([128, 512], F32, tag="pobc",
                                              name="bc")
                                nc.tensor.matmul(
                                    bc[0:64, :], onesT[0:1, 0:64], rAb[:],
                                    start=True, stop=True)
                                nc.tensor.matmul(
                                    bc[64:128, :], onesT[0:1, 0:64], rBb[:],
                                    start=True, stop=True)
                                nc.vector.tensor_mul(
                                    atn[p][:, qf], atn[p][:, qf], bc[:])
                            tail_q.append(norm_p)
                        else:
                            nc.sync.dma_start(
                                sums_qc[2 * p:2 * p + 1, :], stgA[:])
                            nc.sync.dma_start(
                                sums_qc[2 * p + 1:2 * p + 2, :], stgB[:])
                return op

            for qc in range(NQ):
                qf = slice(512 * qc, 512 * (qc + 1))
                jmax = 4 * qc + 3
                sums_qc = smp.tile([8, 512], F32, tag="sums", name="sums")
                for p in range(NT):
                    ATA = atp.tile([65, 512], F32, tag="at", name="ATA")
                    ATB = atp.tile([65, 512], F32, tag="at", name="ATB")
                    for j in range(jmax + 1):
                        off = max(0, 128 * j - 512 * qc)
                        qs = slice(512 * qc + off, 512 * (qc + 1))
                        SC = scp.tile([128, 1024], F32, tag="sc", name="SC")
                        nc.tensor.matmul(
                            SC[:, off:512],
                            kwT[p][0:64, 128 * j:128 * (j + 1)],
                            qwT[p][0:64, qs],
                            start=True, stop=True, tile_position=(0, 0))
                        nc.tensor.matmul(
                            SC[:, 512 + off:1024],
                            kwT[p][64:128, 128 * j:128 * (j + 1)],
                            qwT[p][64:128, qs],
                            start=True, stop=True, tile_position=(64, 0))
                        PR = prp.tile([128, 1024], BF16, tag="pr", name="PR")
                        if off == 0 and j < 4 * qc:
                            nc.scalar.activation(PR[:], SC[:], EXP, scale=1.0 / 8.0)
                        else:
                            nc.scalar.activation(PR[:, off:512], SC[:, off:512],
                                                 EXP, scale=1.0 / 8.0)
                            nc.scalar.activation(PR[:, 512 + off:1024],
                                                 SC[:, 512 + off:1024],
                                                 EXP, scale=1.0 / 8.0)
                            for so in (off, 512 + off):
                                nc.gpsimd.affine_select(
                                    out=PR[:, so:so + 128],
                                    in_=PR[:, so:so + 128],
                                    channel_multiplier=-1,
                                    pattern=[[1, 128]], base=0,
                                    compare_op=mybir.AluOpType.is_ge,
                                    fill=0.0)
                        if len(pv_lag) >= 3:
                            pv_lag.popleft()()
                        pv_lag.append(mk_pv(qc, p, j, jmax, off, ATA, ATB, PR,
                                            sums_qc, qf))
                        # drain pending tail ops from the previous qc: every
                        # other j, and not in the first few j's, so the recip
                        # (DVE) finishes before the first bc matmul reaches
                        # the PE queue head
                        if tail_q and j % 2 == 1 and (p > 0 or j >= 5):
                            tail_q.popleft()()
                while tail_q:
                    tail_q.popleft()()
                if qc < NQ - 1:
                    tail_q.extend(make_tail(qc, sums_qc))
            # flush remaining PVs (incl. the last pair's drain + norm_p)
            # BEFORE queueing the final output projections, which read every
            # normalized atn tile
            while pv_lag:
                pv_lag.popleft()()
            tail_q.extend(make_tail(NQ - 1, None))
            while tail_q:
                tail_q.popleft()()

    nc.compile()
    return nc


_NC_CACHE = {}


def get_nc():
    if "nc" not in _NC_CACHE:
        _NC_CACHE["nc"] = build_nc()
    return _NC_CACHE["nc"]


def make_in_maps(q, k, v, Wq, bq, Wk, bk, Wv, bv, Wo):
    """Host-side shard prep. Returns list of 8 per-core input dicts."""
    f = np.float32
    bf = ml_dtypes.bfloat16
    q = np.asarray(q, f)
    k = np.asarray(k, f)
    v = np.asarray(v, f)
    Wq, bq = np.asarray(Wq, f), np.asarray(bq, f)
    Wk, bk = np.asarray(Wk, f), np.asarray(bk, f)
    Wv, bv = np.asarray(Wv, f), np.asarray(bv, f)
    Wo = np.asarray(Wo, f)
    E = np.zeros((8, DL), f)
    for h in range(8):
        E[h, 64 * h:64 * (h + 1)] = 1.0
    qT = [np.ascontiguousarray(q[b].T).astype(bf) for b in range(B)]
    kT = [np.ascontiguousarray(k[b].T).astype(bf) for b in range(B)]
    vT = [np.ascontiguousarray(v[b].T).astype(bf) for b in range(B)]
    in_maps = []
    for c in range(8):
        b, g = c // 2, c % 2
        cs = slice(DL * g, DL * (g + 1))
        in_maps.append(dict(
            qT=qT[b],
            kT=kT[b],
            vT=vT[b],
            Wq_s=np.ascontiguousarray(Wq[:, cs]).astype(bf),
            Wk_s=np.ascontiguousarray(Wk[:, cs]).astype(bf),
            Wv_s=np.ascontiguousarray(Wv[:, cs]).astype(bf),
            Wo_s=np.ascontiguousarray(Wo[cs, :]).astype(bf),
            bq_s=np.ascontiguousarray(bq[cs]).reshape(DL, 1),
            bk_s=np.ascontiguousarray(bk[cs]).reshape(DL, 1),
            bv_bc=np.tile(bv[cs][None, :], (128, 1)).astype(bf),
            E_in=E.astype(bf),
        ))
    return in_maps


def unshard(results, bo):
    bo = np.asarray(bo, np.float32)
    out = np.empty((B, S, D), np.float32)
    for b in range(B):
        pT = (results[2 * b]["out_partial"].astype(np.float32)
              + results[2 * b + 1]["out_partial"].astype(np.float32))
        out[b] = pT.T + bo
    return out


def kernel(q, k, v, mask, Wq, bq, Wk, bk, Wv, bv, Wo, bo, **_unused):
    nc = get_nc()
    in_maps = make_in_maps(q, k, v, Wq, bq, Wk, bk, Wv, bv, Wo)
    res = run_bass_kernel_spmd(nc, in_maps, core_ids=list(range(8))).results
    return unshard(res, bo)
